# revision 16
# baseline (speedup 1.0000x reference)
"""AdditiveAttention on Trainium2 (Bass/Tile), data-parallel over batch across 8 cores.

Per-core problem (batch element b on core b):
  q = queries @ Wq                  (128, 256)
  k = keys @ Wk                     (512, 256)
  scores[i,j] = wv . tanh(q[i] + k[j])          (128, 512)
  masked softmax over j (j >= valid_len -> -1e6)
  out = attn @ values               (128, 256)

Kernel structure per core:
  - q/k projected in fp32, then split hi/lo into bf16 pairs (q ~ q_hi + q_lo)
  - PE K=4 bf16 matmuls build preact_h[i,j] = q[i,h] + k[j,h] in PSUM
    (rows: q_hi, q_lo, ones | ones, ones, k_hi, k_lo -> exact to ~2^-17)
  - ACT applies tanh on groups of 3 banks (PSUM -> bf16 SBUF)
  - DVE prescales feat by wv_h (bf16 4x mode, fp32 per-partition scalar)
  - PE accumulates scores += I.T @ (wv_h * feat_h) in PSUM (bf16 identity)
  - mask row added via rank-1 matmul (host-computed from valid_lens)
  - softmax: DVE reduce_max(neg) -> ACT exp(bias=-max, accum_out=sumexp) -> recip
  - PE transposes attn, 4 accumulating fp32 matmuls against values, row-scale by 1/sum
"""

import numpy as np
import ml_dtypes
from contextlib import ExitStack

from concourse import bacc, tile
import concourse.bass as bass
import concourse.mybir as mybir
from concourse.bass_utils import run_bass_kernel_spmd

F32 = mybir.dt.float32
BF16 = mybir.dt.bfloat16
AF = mybir.ActivationFunctionType
ts = bass.ts

Lq, Lk, D, H = 128, 512, 256, 256
NCORES = 8
CH = 8    # h-values per staged SBUF chunk
GRP = 3   # h-values per tanh group (3 PSUM banks)

_CACHE = {}


def build_program():
    nc = bacc.Bacc(
        "TRN2", target_bir_lowering=False, debug=False, enable_asserts=False
    )

    queries_d = nc.dram_tensor("queries", [Lq, D], F32, kind="ExternalInput")
    keys_d = nc.dram_tensor("keys", [Lk, D], F32, kind="ExternalInput")
    values_d = nc.dram_tensor("values", [Lk, H], F32, kind="ExternalInput")
    Wq_d = nc.dram_tensor("Wq", [D, H], F32, kind="ExternalInput")
    Wk_d = nc.dram_tensor("Wk", [D, H], F32, kind="ExternalInput")
    wv_d = nc.dram_tensor("wv", [1, H], F32, kind="ExternalInput")
    mask_d = nc.dram_tensor("mask", [1, Lk], BF16, kind="ExternalInput")
    ones_d = nc.dram_tensor("ones", [2, CH * Lk], BF16, kind="ExternalInput")
    ident_d = nc.dram_tensor("ident", [128, 128], F32, kind="ExternalInput")
    identb_d = nc.dram_tensor("identb", [128, 128], BF16, kind="ExternalInput")
    wvdiag_d = nc.dram_tensor("wvdiag", [H * 128, 128], BF16, kind="ExternalInput")
    out_d = nc.dram_tensor("out", [Lq, H], F32, kind="ExternalOutput")

    with tile.TileContext(nc) as tc, ExitStack() as ctx:
        const = ctx.enter_context(tc.tile_pool(name="const", bufs=1))
        inp = ctx.enter_context(tc.tile_pool(name="inp", bufs=1))
        proj = ctx.enter_context(tc.tile_pool(name="proj", bufs=1))
        kch = ctx.enter_context(tc.tile_pool(name="kch", bufs=3))
        qch = ctx.enter_context(tc.tile_pool(name="qch", bufs=3))
        featp = ctx.enter_context(tc.tile_pool(name="featp", bufs=2))
        diagp = ctx.enter_context(tc.tile_pool(name="diagp", bufs=8))
        sm = ctx.enter_context(tc.tile_pool(name="sm", bufs=1))
        ps_pre = ctx.enter_context(tc.tile_pool(name="ps_pre", bufs=2, space="PSUM"))
        ps_sc = ctx.enter_context(tc.tile_pool(name="ps_sc", bufs=1, space="PSUM"))
        ps_misc = ctx.enter_context(tc.tile_pool(name="ps_misc", bufs=1, space="PSUM"))

        # ---- input loads (ident first: transposes need it; values last) ----
        ident_sb = const.tile([128, 128], F32)
        nc.sync.dma_start(ident_sb[:], ident_d[:])
        # preload the ACT spline tables (tanh/exp) off the critical path
        warm_sb = sm.tile([1, 2], F32)
        nc.scalar.activation(warm_sb[0:1, 0:1], ident_sb[0:1, 0:1], AF.Tanh)
        nc.scalar.activation(warm_sb[0:1, 1:2], ident_sb[0:1, 0:1], AF.Exp)
        queries_sb = inp.tile([128, D], F32)
        nc.sync.dma_start(queries_sb[:], queries_d[:])
        Wq_sb = inp.tile([128, 2 * H], F32)  # [d', dt*256 + h]
        for dt in range(2):
            nc.sync.dma_start(Wq_sb[:, ts(dt, H)], Wq_d[ts(dt, 128), :])
        keys_sb = inp.tile([128, 4 * D], F32)  # [j', jt*256 + d]
        for jt in range(4):
            nc.sync.dma_start(keys_sb[:, ts(jt, D)], keys_d[ts(jt, 128), :])
        Wk_sb = inp.tile([128, 2 * H], F32)
        for dt in range(2):
            nc.gpsimd.dma_start(Wk_sb[:, ts(dt, H)], Wk_d[ts(dt, 128), :])
        identb_sb = const.tile([128, 128], BF16)
        nc.gpsimd.dma_start(identb_sb[:], identb_d[:])
        mask_sb = const.tile([1, Lk], BF16)
        nc.gpsimd.dma_start(mask_sb[:], mask_d[:])
        onesb_sb = const.tile([1, 128], BF16)
        nc.gpsimd.dma_start(onesb_sb[:], ones_d[0:1, 0:128])
        values_sb = inp.tile([128, 4 * H], F32)  # [j', jt*256 + v]
        for jt in range(4):
            nc.gpsimd.dma_start(values_sb[:, ts(jt, H)], values_d[ts(jt, 128), :])

        # Manual ring of K=128 zero-padded chunk tiles. Rows 0-3 carry the
        # rank-4 operands; rows 4-127 stay zero so every preact matmul drives
        # the full 128-row array (keeps the PE activity monitor warm).
        NRING = 5
        krings = []
        qrings = []
        for r in range(NRING):
            kr = kch.tile([128, CH * Lk], BF16, tag=f"kring{r}")
            eng = nc.gpsimd if r % 2 == 0 else nc.vector
            eng.memset(kr[:, :], 0.0)
            nc.sync.dma_start(kr[0:2, :], ones_d[:, 0 : CH * Lk])
            krings.append(kr)
            qr = qch.tile([128, CH * 128], BF16, tag=f"qring{r}")
            nc.vector.memset(qr[:, :], 0.0)
            nc.sync.dma_start(qr[2:4, :], ones_d[:, 0 : CH * 128])
            qrings.append(qr)

        # ---- transposes: queries -> qsT (d, i); keys -> keysT (d, j) ----
        qsT_ps = ps_misc.tile([128, D], F32, tag="misc")
        for dt in range(2):
            nc.tensor.transpose(
                qsT_ps[:, ts(dt, 128)], queries_sb[:, ts(dt, 128)], ident_sb[:]
            )
        qsT_sb = proj.tile([128, D], F32)  # [d', dt*128 + i]
        nc.vector.tensor_copy(qsT_sb[:], qsT_ps[:])

        keysT_sb = proj.tile([128, 2 * Lk], F32)  # [d', dt*512 + jt*128 + j']
        for dt in range(2):
            keysT_ps = ps_pre.tile([128, GRP * Lk], F32, tag="pre")
            for jt in range(4):
                nc.tensor.transpose(
                    keysT_ps[:, ts(jt, 128)],
                    keys_sb[:, jt * D + dt * 128 : jt * D + dt * 128 + 128],
                    ident_sb[:],
                )
            nc.vector.tensor_copy(keysT_sb[:, ts(dt, Lk)], keysT_ps[:, 0:Lk])

        # ---- projections (fp32): qT (h, i), kT (h, j); split hi/lo bf16 ----
        qT_ps = ps_misc.tile([128, D], F32, tag="misc")
        for ht in range(2):
            for dt in range(2):
                nc.tensor.matmul(
                    qT_ps[:, ts(ht, 128)],
                    Wq_sb[:, dt * H + ht * 128 : dt * H + ht * 128 + 128],
                    qsT_sb[:, ts(dt, 128)],
                    start=(dt == 0),
                    stop=(dt == 1),
                )
        qhi_sb = proj.tile([128, D], BF16)  # [h', ht*128 + i]
        nc.vector.tensor_copy(qhi_sb[:], qT_ps[:])
        qlo_sb = proj.tile([128, D], BF16)
        nc.vector.tensor_sub(qlo_sb[:], qT_ps[:], qhi_sb[:])

        khi_sb = proj.tile([128, 2 * Lk], BF16)  # [h', ht*512 + j]
        klo_sb = proj.tile([128, 2 * Lk], BF16)
        for ht in range(2):
            kT_ps = ps_pre.tile([128, GRP * Lk], F32, tag="pre")
            for dt in range(2):
                nc.tensor.matmul(
                    kT_ps[:, 0:Lk],
                    Wk_sb[:, dt * H + ht * 128 : dt * H + ht * 128 + 128],
                    keysT_sb[:, ts(dt, Lk)],
                    start=(dt == 0),
                    stop=(dt == 1),
                )
            nc.vector.tensor_copy(khi_sb[:, ts(ht, Lk)], kT_ps[:, 0:Lk])
            nc.vector.tensor_sub(
                klo_sb[:, ts(ht, Lk)], kT_ps[:, 0:Lk], khi_sb[:, ts(ht, Lk)]
            )

        def row_src(sb, c, width):
            # rows h = c*CH .. c*CH+CH of the (h, x) matrix stored as
            # sb[(h % 128), (h // 128)*width + x] -> (CH, width) slice; the
            # DMA flattens it row-major into the chunk row (same total size)
            ht, r0 = divmod(c * CH, 128)
            return sb[r0 : r0 + CH, ts(ht, width)]

        loaded = {}

        def get_chunks(c):
            # kc rows: [ones, ones, k_hi, k_lo]; qc rows: [q_hi, q_lo, ones, ones]
            if c not in loaded:
                kc = krings[c % NRING]
                nc.sync.dma_start(kc[2:3, :], row_src(khi_sb, c, Lk))
                nc.sync.dma_start(kc[3:4, :], row_src(klo_sb, c, Lk))
                qc = qrings[c % NRING]
                nc.sync.dma_start(qc[0:1, :], row_src(qhi_sb, c, 128))
                nc.sync.dma_start(qc[1:2, :], row_src(qlo_sb, c, 128))
                loaded[c] = (qc, kc)
            return loaded[c]

        # ---- scores accumulator; masked init via rank-1 matmul ----
        sc_ps = ps_sc.tile([128, Lk], F32)
        nc.tensor.matmul(sc_ps[:], onesb_sb[:], mask_sb[:], start=True, stop=False)

        # ---- main loop over h in groups of GRP (software-pipelined) ----
        # PE FIFO order matters: preact matmuls for group g+2 are emitted
        # BEFORE the accumulate matmuls of group g, so the tanh for g+1 never
        # transitively waits on the DVE->accumulate chain.
        n_groups = (H + GRP - 1) // GRP
        n_chunks = H // CH
        pres = {}

        diags = {}

        def emit_pre(g):
            h0 = g * GRP
            sz = min(GRP, H - h0)
            for pc in range(h0 // CH, min((h0 + sz - 1) // CH + 2, n_chunks)):
                get_chunks(pc)
            dgs = []
            diag_engs = (nc.gpsimd, nc.sync, nc.gpsimd)
            for l in range(sz):
                h = h0 + l
                dg = diagp.tile([128, 128], BF16, tag="dg")
                diag_engs[l % 3].dma_start(dg[:], wvdiag_d[ts(h, 128), :])
                dgs.append(dg)
            diags[g] = dgs
            pre = ps_pre.tile([128, GRP * Lk], F32, tag="pre")
            for l in range(sz):
                h = h0 + l
                c, hh = divmod(h, CH)
                qc, kc = get_chunks(c)
                nc.tensor.matmul(
                    pre[:, ts(l, Lk)],
                    qc[:, ts(hh, 128)],
                    kc[:, ts(hh, Lk)],
                    start=True,
                    stop=True,
                )
            pres[g] = pre

        emit_pre(0)
        emit_pre(1)
        for g in range(n_groups):
            h0 = g * GRP
            sz = min(GRP, H - h0)
            pre = pres.pop(g)
            feat = featp.tile([128, GRP * Lk], BF16, tag="feat")
            nc.scalar.activation(feat[:, 0 : sz * Lk], pre[:, 0 : sz * Lk], AF.Tanh)
            if g + 2 < n_groups:
                emit_pre(g + 2)
            dgs = diags.pop(g)
            for l in range(sz):
                h = h0 + l
                nc.tensor.matmul(
                    sc_ps[:],
                    dgs[l][:],
                    feat[:, ts(l, Lk)],
                    start=False,
                    stop=(h == H - 1),
                )

        # ---- masked softmax over j ----
        negmax = sm.tile([128, 1], F32)
        nc.vector.tensor_reduce(
            negmax[:],
            sc_ps[:],
            axis=mybir.AxisListType.X,
            op=mybir.AluOpType.max,
            negate=True,
        )
        p_sb = sm.tile([128, Lk], F32)
        sumexp = sm.tile([128, 1], F32)
        nc.scalar.activation(
            p_sb[:], sc_ps[:], AF.Exp, bias=negmax[:], accum_out=sumexp[:]
        )
        rinv = sm.tile([128, 1], F32)
        nc.vector.reciprocal(rinv[:], sumexp[:])

        # ---- attn @ values: transpose p, 4 accumulating fp32 matmuls ----
        pT_ps = ps_misc.tile([128, Lk], F32, tag="misc")
        for jt in range(4):
            nc.tensor.transpose(
                pT_ps[:, ts(jt, 128)], p_sb[:, ts(jt, 128)], ident_sb[:]
            )
        pT_sb = sm.tile([128, Lk], F32)  # [j', jt*128 + i]
        nc.vector.tensor_copy(pT_sb[:], pT_ps[:])

        out_ps = ps_misc.tile([128, H], F32, tag="misc")
        for jt in range(4):
            nc.tensor.matmul(
                out_ps[:],
                pT_sb[:, ts(jt, 128)],
                values_sb[:, ts(jt, H)],
                start=(jt == 0),
                stop=(jt == 3),
            )
        out_sb = sm.tile([128, H], F32)
        nc.vector.tensor_scalar_mul(out_sb[:], out_ps[:], rinv[:])
        nc.sync.dma_start(out_d[:], out_sb[:])

    nc.compile()
    return nc


def _get_program():
    if "nc" not in _CACHE:
        _CACHE["nc"] = build_program()
    return _CACHE["nc"]


def make_in_maps(queries, keys, values, valid_lens, Wq, Wk, wv):
    queries = np.ascontiguousarray(queries, dtype=np.float32)
    keys = np.ascontiguousarray(keys, dtype=np.float32)
    values = np.ascontiguousarray(values, dtype=np.float32)
    Wq = np.ascontiguousarray(Wq, dtype=np.float32)
    Wk = np.ascontiguousarray(Wk, dtype=np.float32)
    wv = np.ascontiguousarray(wv, dtype=np.float32).reshape(1, H)
    vl = np.asarray(valid_lens).astype(np.int64).reshape(NCORES)
    bf = ml_dtypes.bfloat16
    ones = np.ones((2, CH * Lk), dtype=bf)
    ident = np.eye(128, dtype=np.float32)
    identb = np.eye(128, dtype=bf)
    wvdiag = np.zeros((H, 128, 128), dtype=bf)
    idx = np.arange(128)
    for h in range(H):
        wvdiag[h, idx, idx] = bf(wv[0, h])
    wvdiag = wvdiag.reshape(H * 128, 128)
    jj = np.arange(Lk)
    in_maps = []
    for b in range(NCORES):
        mask_b = np.where(jj >= vl[b], -1e6, 0.0).astype(bf)[None, :]
        in_maps.append(
            {
                "queries": queries[b],
                "keys": keys[b],
                "values": values[b],
                "Wq": Wq,
                "Wk": Wk,
                "wv": wv,
                "mask": mask_b,
                "ones": ones,
                "ident": ident,
                "identb": identb,
                "wvdiag": wvdiag,
            }
        )
    return in_maps


def kernel(**inputs):
    nc = _get_program()
    in_maps = make_in_maps(
        inputs["queries"],
        inputs["keys"],
        inputs["values"],
        inputs["valid_lens"],
        inputs["Wq"],
        inputs["Wk"],
        inputs["wv"],
    )
    res = run_bass_kernel_spmd(nc, in_maps, core_ids=list(range(NCORES)))
    out = np.stack([res.results[b]["out"] for b in range(NCORES)], axis=0)
    return out.astype(np.float32)


# revision 17
# speedup vs baseline: 1.5449x; 1.5449x over previous
"""AdditiveAttention on Trainium2 (Bass/Tile), data-parallel over batch across 8 cores.

Per-core problem (batch element b on core b):
  q = queries @ Wq                  (128, 256)
  k = keys @ Wk                     (512, 256)
  scores[i,j] = wv . tanh(q[i] + k[j])          (128, 512)
  masked softmax over j (j >= valid_len -> -1e6)
  out = attn @ values               (128, 256)

Kernel structure per core:
  - q/k projected in fp32, then split hi/lo into bf16 pairs (q ~ q_hi + q_lo)
  - PE K=4 bf16 matmuls build preact_h[i,j] = q[i,h] + k[j,h] in PSUM
    (rows: q_hi, q_lo, ones | ones, ones, k_hi, k_lo -> exact to ~2^-17)
  - ACT applies tanh on groups of 3 banks (PSUM -> bf16 SBUF)
  - DVE prescales feat by wv_h (bf16 4x mode, fp32 per-partition scalar)
  - PE accumulates scores += I.T @ (wv_h * feat_h) in PSUM (bf16 identity)
  - mask row added via rank-1 matmul (host-computed from valid_lens)
  - softmax: DVE reduce_max(neg) -> ACT exp(bias=-max, accum_out=sumexp) -> recip
  - PE transposes attn, 4 accumulating fp32 matmuls against values, row-scale by 1/sum
"""

import numpy as np
import ml_dtypes
from contextlib import ExitStack

from concourse import bacc, tile
import concourse.bass as bass
import concourse.mybir as mybir
from concourse.bass_utils import run_bass_kernel_spmd

F32 = mybir.dt.float32
BF16 = mybir.dt.bfloat16
AF = mybir.ActivationFunctionType
ts = bass.ts

Lq, Lk, D, H = 128, 512, 256, 256
NCORES = 8
CH = 8    # h-values per staged SBUF chunk
GRP = 3   # h-values per tanh group (3 PSUM banks)

_CACHE = {}


def build_program():
    nc = bacc.Bacc(
        "TRN2", target_bir_lowering=False, debug=False, enable_asserts=False
    )

    queries_d = nc.dram_tensor("queries", [Lq, D], F32, kind="ExternalInput")
    keys_d = nc.dram_tensor("keys", [Lk, D], F32, kind="ExternalInput")
    values_d = nc.dram_tensor("values", [Lk, H], F32, kind="ExternalInput")
    Wq_d = nc.dram_tensor("Wq", [D, H], F32, kind="ExternalInput")
    Wk_d = nc.dram_tensor("Wk", [D, H], F32, kind="ExternalInput")
    wv_d = nc.dram_tensor("wv", [1, H], F32, kind="ExternalInput")
    mask_d = nc.dram_tensor("mask", [1, Lk], BF16, kind="ExternalInput")
    ones_d = nc.dram_tensor("ones", [2, CH * Lk], BF16, kind="ExternalInput")
    ident_d = nc.dram_tensor("ident", [128, 128], F32, kind="ExternalInput")
    identb_d = nc.dram_tensor("identb", [128, 128], BF16, kind="ExternalInput")
    wvdiag_d = nc.dram_tensor("wvdiag", [(H // GRP + 1) * 128, GRP * 128], BF16, kind="ExternalInput")
    out_d = nc.dram_tensor("out", [Lq, H], F32, kind="ExternalOutput")

    with tile.TileContext(nc) as tc, ExitStack() as ctx:
        const = ctx.enter_context(tc.tile_pool(name="const", bufs=1))
        inp = ctx.enter_context(tc.tile_pool(name="inp", bufs=1))
        proj = ctx.enter_context(tc.tile_pool(name="proj", bufs=1))
        kch = ctx.enter_context(tc.tile_pool(name="kch", bufs=3))
        qch = ctx.enter_context(tc.tile_pool(name="qch", bufs=3))
        featp = ctx.enter_context(tc.tile_pool(name="featp", bufs=2))
        diagp = ctx.enter_context(tc.tile_pool(name="diagp", bufs=4))
        sm = ctx.enter_context(tc.tile_pool(name="sm", bufs=1))
        ps_pre = ctx.enter_context(tc.tile_pool(name="ps_pre", bufs=2, space="PSUM"))
        ps_sc = ctx.enter_context(tc.tile_pool(name="ps_sc", bufs=1, space="PSUM"))
        ps_misc = ctx.enter_context(tc.tile_pool(name="ps_misc", bufs=1, space="PSUM"))

        # ---- input loads (ident first: transposes need it; values last) ----
        ident_sb = const.tile([128, 128], F32)
        nc.sync.dma_start(ident_sb[:], ident_d[:])
        # preload the ACT spline tables (tanh/exp) off the critical path
        warm_sb = sm.tile([1, 2], F32)
        nc.scalar.activation(warm_sb[0:1, 0:1], ident_sb[0:1, 0:1], AF.Tanh)
        nc.scalar.activation(warm_sb[0:1, 1:2], ident_sb[0:1, 0:1], AF.Exp)
        queries_sb = inp.tile([128, D], F32)
        nc.sync.dma_start(queries_sb[:], queries_d[:])
        Wq_sb = inp.tile([128, 2 * H], F32)  # [d', dt*256 + h]
        for dt in range(2):
            nc.sync.dma_start(Wq_sb[:, ts(dt, H)], Wq_d[ts(dt, 128), :])
        keys_sb = inp.tile([128, 4 * D], F32)  # [j', jt*256 + d]
        for jt in range(4):
            nc.sync.dma_start(keys_sb[:, ts(jt, D)], keys_d[ts(jt, 128), :])
        Wk_sb = inp.tile([128, 2 * H], F32)
        for dt in range(2):
            nc.gpsimd.dma_start(Wk_sb[:, ts(dt, H)], Wk_d[ts(dt, 128), :])
        identb_sb = const.tile([128, 128], BF16)
        nc.gpsimd.dma_start(identb_sb[:], identb_d[:])
        mask_sb = const.tile([1, Lk], BF16)
        nc.gpsimd.dma_start(mask_sb[:], mask_d[:])
        onesb_sb = const.tile([1, 128], BF16)
        nc.gpsimd.dma_start(onesb_sb[:], ones_d[0:1, 0:128])
        values_sb = inp.tile([128, 4 * H], F32)  # [j', jt*256 + v]
        for jt in range(4):
            nc.gpsimd.dma_start(values_sb[:, ts(jt, H)], values_d[ts(jt, 128), :])

        # Manual ring of K=128 zero-padded chunk tiles. Rows 0-3 carry the
        # rank-4 operands; rows 4-127 stay zero so every preact matmul drives
        # the full 128-row array (keeps the PE activity monitor warm).
        NRING = 5
        krings = []
        qrings = []
        for r in range(NRING):
            kr = kch.tile([128, CH * Lk], BF16, tag=f"kring{r}")
            eng = nc.gpsimd if r % 2 == 0 else nc.vector
            eng.memset(kr[:, :], 0.0)
            nc.sync.dma_start(kr[0:2, :], ones_d[:, 0 : CH * Lk])
            krings.append(kr)
            qr = qch.tile([128, CH * 128], BF16, tag=f"qring{r}")
            nc.vector.memset(qr[:, :], 0.0)
            nc.sync.dma_start(qr[2:4, :], ones_d[:, 0 : CH * 128])
            qrings.append(qr)

        # ---- transposes: queries -> qsT (d, i); keys -> keysT (d, j) ----
        qsT_ps = ps_misc.tile([128, D], F32, tag="misc")
        for dt in range(2):
            nc.tensor.transpose(
                qsT_ps[:, ts(dt, 128)], queries_sb[:, ts(dt, 128)], ident_sb[:]
            )
        qsT_sb = proj.tile([128, D], F32)  # [d', dt*128 + i]
        nc.vector.tensor_copy(qsT_sb[:], qsT_ps[:])

        keysT_sb = proj.tile([128, 2 * Lk], F32)  # [d', dt*512 + jt*128 + j']
        for dt in range(2):
            keysT_ps = ps_pre.tile([128, GRP * Lk], F32, tag="pre")
            for jt in range(4):
                nc.tensor.transpose(
                    keysT_ps[:, ts(jt, 128)],
                    keys_sb[:, jt * D + dt * 128 : jt * D + dt * 128 + 128],
                    ident_sb[:],
                )
            nc.vector.tensor_copy(keysT_sb[:, ts(dt, Lk)], keysT_ps[:, 0:Lk])

        # ---- projections (fp32): qT (h, i), kT (h, j); split hi/lo bf16 ----
        qT_ps = ps_misc.tile([128, D], F32, tag="misc")
        for ht in range(2):
            for dt in range(2):
                nc.tensor.matmul(
                    qT_ps[:, ts(ht, 128)],
                    Wq_sb[:, dt * H + ht * 128 : dt * H + ht * 128 + 128],
                    qsT_sb[:, ts(dt, 128)],
                    start=(dt == 0),
                    stop=(dt == 1),
                )
        qhi_sb = proj.tile([128, D], BF16)  # [h', ht*128 + i]
        nc.vector.tensor_copy(qhi_sb[:], qT_ps[:])
        qlo_sb = proj.tile([128, D], BF16)
        nc.vector.tensor_sub(qlo_sb[:], qT_ps[:], qhi_sb[:])

        khi_sb = proj.tile([128, 2 * Lk], BF16)  # [h', ht*512 + j]
        klo_sb = proj.tile([128, 2 * Lk], BF16)
        for ht in range(2):
            kT_ps = ps_pre.tile([128, GRP * Lk], F32, tag="pre")
            for dt in range(2):
                nc.tensor.matmul(
                    kT_ps[:, 0:Lk],
                    Wk_sb[:, dt * H + ht * 128 : dt * H + ht * 128 + 128],
                    keysT_sb[:, ts(dt, Lk)],
                    start=(dt == 0),
                    stop=(dt == 1),
                )
            nc.vector.tensor_copy(khi_sb[:, ts(ht, Lk)], kT_ps[:, 0:Lk])
            nc.vector.tensor_sub(
                klo_sb[:, ts(ht, Lk)], kT_ps[:, 0:Lk], khi_sb[:, ts(ht, Lk)]
            )

        def row_src(sb, c, width):
            # rows h = c*CH .. c*CH+CH of the (h, x) matrix stored as
            # sb[(h % 128), (h // 128)*width + x] -> (CH, width) slice; the
            # DMA flattens it row-major into the chunk row (same total size)
            ht, r0 = divmod(c * CH, 128)
            return sb[r0 : r0 + CH, ts(ht, width)]

        loaded = {}

        def get_chunks(c):
            # kc rows: [ones, ones, k_hi, k_lo]; qc rows: [q_hi, q_lo, ones, ones]
            if c not in loaded:
                kc = krings[c % NRING]
                nc.sync.dma_start(kc[2:3, :], row_src(khi_sb, c, Lk))
                nc.sync.dma_start(kc[3:4, :], row_src(klo_sb, c, Lk))
                qc = qrings[c % NRING]
                nc.sync.dma_start(qc[0:1, :], row_src(qhi_sb, c, 128))
                nc.sync.dma_start(qc[1:2, :], row_src(qlo_sb, c, 128))
                loaded[c] = (qc, kc)
            return loaded[c]

        # ---- scores accumulator; masked init via rank-1 matmul ----
        sc_ps = ps_sc.tile([128, Lk], F32)
        nc.tensor.matmul(sc_ps[:], onesb_sb[:], mask_sb[:], start=True, stop=False)

        # ---- main loop over h in groups of GRP (software-pipelined) ----
        # PE FIFO order matters: preact matmuls for group g+2 are emitted
        # BEFORE the accumulate matmuls of group g, so the tanh for g+1 never
        # transitively waits on the DVE->accumulate chain.
        n_groups = (H + GRP - 1) // GRP
        n_chunks = H // CH
        pres = {}

        diags = {}

        def emit_pre(g):
            h0 = g * GRP
            sz = min(GRP, H - h0)
            for pc in range(h0 // CH, min((h0 + sz - 1) // CH + 2, n_chunks)):
                get_chunks(pc)
            dg3 = diagp.tile([128, GRP * 128], BF16, tag="dg")
            nc.gpsimd.dma_start(dg3[:], wvdiag_d[ts(g, 128), :])
            diags[g] = dg3
            pre = ps_pre.tile([128, GRP * Lk], F32, tag="pre")
            for l in range(sz):
                h = h0 + l
                c, hh = divmod(h, CH)
                qc, kc = get_chunks(c)
                nc.tensor.matmul(
                    pre[:, ts(l, Lk)],
                    qc[:, ts(hh, 128)],
                    kc[:, ts(hh, Lk)],
                    start=True,
                    stop=True,
                )
            pres[g] = pre

        emit_pre(0)
        emit_pre(1)
        for g in range(n_groups):
            h0 = g * GRP
            sz = min(GRP, H - h0)
            pre = pres.pop(g)
            feat = featp.tile([128, GRP * Lk], BF16, tag="feat")
            nc.scalar.activation(feat[:, 0 : sz * Lk], pre[:, 0 : sz * Lk], AF.Tanh)
            if g + 2 < n_groups:
                emit_pre(g + 2)
            dg3 = diags.pop(g)
            for l in range(sz):
                h = h0 + l
                nc.tensor.matmul(
                    sc_ps[:],
                    dg3[:, ts(l, 128)],
                    feat[:, ts(l, Lk)],
                    start=False,
                    stop=(h == H - 1),
                )

        # ---- masked softmax over j ----
        negmax = sm.tile([128, 1], F32)
        nc.vector.tensor_reduce(
            negmax[:],
            sc_ps[:],
            axis=mybir.AxisListType.X,
            op=mybir.AluOpType.max,
            negate=True,
        )
        p_sb = sm.tile([128, Lk], F32)
        sumexp = sm.tile([128, 1], F32)
        nc.scalar.activation(
            p_sb[:], sc_ps[:], AF.Exp, bias=negmax[:], accum_out=sumexp[:]
        )
        rinv = sm.tile([128, 1], F32)
        nc.vector.reciprocal(rinv[:], sumexp[:])

        # ---- attn @ values: transpose p, 4 accumulating fp32 matmuls ----
        pT_ps = ps_misc.tile([128, Lk], F32, tag="misc")
        for jt in range(4):
            nc.tensor.transpose(
                pT_ps[:, ts(jt, 128)], p_sb[:, ts(jt, 128)], ident_sb[:]
            )
        pT_sb = sm.tile([128, Lk], F32)  # [j', jt*128 + i]
        nc.vector.tensor_copy(pT_sb[:], pT_ps[:])

        out_ps = ps_misc.tile([128, H], F32, tag="misc")
        for jt in range(4):
            nc.tensor.matmul(
                out_ps[:],
                pT_sb[:, ts(jt, 128)],
                values_sb[:, ts(jt, H)],
                start=(jt == 0),
                stop=(jt == 3),
            )
        out_sb = sm.tile([128, H], F32)
        nc.vector.tensor_scalar_mul(out_sb[:], out_ps[:], rinv[:])
        nc.sync.dma_start(out_d[:], out_sb[:])

    nc.compile()
    return nc


def _get_program():
    if "nc" not in _CACHE:
        _CACHE["nc"] = build_program()
    return _CACHE["nc"]


def make_in_maps(queries, keys, values, valid_lens, Wq, Wk, wv):
    queries = np.ascontiguousarray(queries, dtype=np.float32)
    keys = np.ascontiguousarray(keys, dtype=np.float32)
    values = np.ascontiguousarray(values, dtype=np.float32)
    Wq = np.ascontiguousarray(Wq, dtype=np.float32)
    Wk = np.ascontiguousarray(Wk, dtype=np.float32)
    wv = np.ascontiguousarray(wv, dtype=np.float32).reshape(1, H)
    vl = np.asarray(valid_lens).astype(np.int64).reshape(NCORES)
    bf = ml_dtypes.bfloat16
    ones = np.ones((2, CH * Lk), dtype=bf)
    ident = np.eye(128, dtype=np.float32)
    identb = np.eye(128, dtype=bf)
    n_groups = H // GRP + 1
    wvdiag = np.zeros((n_groups, 128, GRP, 128), dtype=bf)
    idx = np.arange(128)
    for g in range(n_groups):
        for l in range(GRP):
            h = g * GRP + l
            if h < H:
                wvdiag[g, idx, l, idx] = bf(wv[0, h])
    wvdiag = wvdiag.reshape(n_groups * 128, GRP * 128)
    jj = np.arange(Lk)
    in_maps = []
    for b in range(NCORES):
        mask_b = np.where(jj >= vl[b], -1e6, 0.0).astype(bf)[None, :]
        in_maps.append(
            {
                "queries": queries[b],
                "keys": keys[b],
                "values": values[b],
                "Wq": Wq,
                "Wk": Wk,
                "wv": wv,
                "mask": mask_b,
                "ones": ones,
                "ident": ident,
                "identb": identb,
                "wvdiag": wvdiag,
            }
        )
    return in_maps


def kernel(**inputs):
    nc = _get_program()
    in_maps = make_in_maps(
        inputs["queries"],
        inputs["keys"],
        inputs["values"],
        inputs["valid_lens"],
        inputs["Wq"],
        inputs["Wk"],
        inputs["wv"],
    )
    res = run_bass_kernel_spmd(nc, in_maps, core_ids=list(range(NCORES)))
    out = np.stack([res.results[b]["out"] for b in range(NCORES)], axis=0)
    return out.astype(np.float32)


# revision 18
# speedup vs baseline: 1.5763x; 1.0204x over previous
"""AdditiveAttention on Trainium2 (Bass/Tile), data-parallel over batch across 8 cores.

Per-core problem (batch element b on core b):
  q = queries @ Wq                  (128, 256)
  k = keys @ Wk                     (512, 256)
  scores[i,j] = wv . tanh(q[i] + k[j])          (128, 512)
  masked softmax over j (j >= valid_len -> -1e6)
  out = attn @ values               (128, 256)

Kernel structure per core:
  - q/k projected in fp32, then split hi/lo into bf16 pairs (q ~ q_hi + q_lo)
  - PE K=4 bf16 matmuls build preact_h[i,j] = q[i,h] + k[j,h] in PSUM
    (rows: q_hi, q_lo, ones | ones, ones, k_hi, k_lo -> exact to ~2^-17)
  - ACT applies tanh on groups of 3 banks (PSUM -> bf16 SBUF)
  - DVE prescales feat by wv_h (bf16 4x mode, fp32 per-partition scalar)
  - PE accumulates scores += I.T @ (wv_h * feat_h) in PSUM (bf16 identity)
  - mask row added via rank-1 matmul (host-computed from valid_lens)
  - softmax: DVE reduce_max(neg) -> ACT exp(bias=-max, accum_out=sumexp) -> recip
  - PE transposes attn, 4 accumulating fp32 matmuls against values, row-scale by 1/sum
"""

import numpy as np
import ml_dtypes
from contextlib import ExitStack

from concourse import bacc, tile
import concourse.bass as bass
import concourse.mybir as mybir
from concourse.bass_utils import run_bass_kernel_spmd

F32 = mybir.dt.float32
BF16 = mybir.dt.bfloat16
AF = mybir.ActivationFunctionType
ts = bass.ts

Lq, Lk, D, H = 128, 512, 256, 256
NCORES = 8
CH = 8    # h-values per staged SBUF chunk
GRP = 3   # h-values per tanh group (3 PSUM banks)

_CACHE = {}


def build_program():
    nc = bacc.Bacc(
        "TRN2", target_bir_lowering=False, debug=False, enable_asserts=False
    )

    queries_d = nc.dram_tensor("queries", [Lq, D], F32, kind="ExternalInput")
    keys_d = nc.dram_tensor("keys", [Lk, D], F32, kind="ExternalInput")
    values_d = nc.dram_tensor("values", [Lk, H], F32, kind="ExternalInput")
    Wq_d = nc.dram_tensor("Wq", [D, H], F32, kind="ExternalInput")
    Wk_d = nc.dram_tensor("Wk", [D, H], F32, kind="ExternalInput")
    wv_d = nc.dram_tensor("wv", [1, H], F32, kind="ExternalInput")
    mask_d = nc.dram_tensor("mask", [1, Lk], BF16, kind="ExternalInput")
    ones_d = nc.dram_tensor("ones", [2, CH * Lk], BF16, kind="ExternalInput")
    ident_d = nc.dram_tensor("ident", [128, 128], F32, kind="ExternalInput")
    identb_d = nc.dram_tensor("identb", [128, 128], BF16, kind="ExternalInput")
    wvdiag_d = nc.dram_tensor("wvdiag", [(H // GRP + 1) * 128, GRP * 128], BF16, kind="ExternalInput")
    out_d = nc.dram_tensor("out", [Lq, H], F32, kind="ExternalOutput")

    with tile.TileContext(nc) as tc, ExitStack() as ctx:
        const = ctx.enter_context(tc.tile_pool(name="const", bufs=1))
        inp = ctx.enter_context(tc.tile_pool(name="inp", bufs=1))
        proj = ctx.enter_context(tc.tile_pool(name="proj", bufs=1))
        kch = ctx.enter_context(tc.tile_pool(name="kch", bufs=3))
        qch = ctx.enter_context(tc.tile_pool(name="qch", bufs=3))
        featp = ctx.enter_context(tc.tile_pool(name="featp", bufs=4))
        diagp = ctx.enter_context(tc.tile_pool(name="diagp", bufs=4))
        sm = ctx.enter_context(tc.tile_pool(name="sm", bufs=1))
        ps_pre = ctx.enter_context(tc.tile_pool(name="ps_pre", bufs=2, space="PSUM"))
        ps_sc = ctx.enter_context(tc.tile_pool(name="ps_sc", bufs=1, space="PSUM"))
        ps_misc = ctx.enter_context(tc.tile_pool(name="ps_misc", bufs=1, space="PSUM"))

        # ---- input loads (ident first: transposes need it; values last) ----
        ident_sb = const.tile([128, 128], F32)
        nc.sync.dma_start(ident_sb[:], ident_d[:])
        # preload the ACT spline tables (tanh/exp) off the critical path
        warm_sb = sm.tile([1, 2], F32)
        nc.scalar.activation(warm_sb[0:1, 0:1], ident_sb[0:1, 0:1], AF.Tanh)
        nc.scalar.activation(warm_sb[0:1, 1:2], ident_sb[0:1, 0:1], AF.Exp)
        queries_sb = inp.tile([128, D], F32)
        nc.sync.dma_start(queries_sb[:], queries_d[:])
        Wq_sb = inp.tile([128, 2 * H], F32)  # [d', dt*256 + h]
        for dt in range(2):
            nc.sync.dma_start(Wq_sb[:, ts(dt, H)], Wq_d[ts(dt, 128), :])
        keys_sb = inp.tile([128, 4 * D], F32)  # [j', jt*256 + d]
        for jt in range(4):
            nc.sync.dma_start(keys_sb[:, ts(jt, D)], keys_d[ts(jt, 128), :])
        Wk_sb = inp.tile([128, 2 * H], F32)
        for dt in range(2):
            nc.gpsimd.dma_start(Wk_sb[:, ts(dt, H)], Wk_d[ts(dt, 128), :])
        identb_sb = const.tile([128, 128], BF16)
        nc.gpsimd.dma_start(identb_sb[:], identb_d[:])
        mask_sb = const.tile([1, Lk], BF16)
        nc.gpsimd.dma_start(mask_sb[:], mask_d[:])
        onesb_sb = const.tile([1, 128], BF16)
        nc.gpsimd.dma_start(onesb_sb[:], ones_d[0:1, 0:128])
        values_sb = inp.tile([128, 4 * H], F32)  # [j', jt*256 + v]
        for jt in range(4):
            nc.gpsimd.dma_start(values_sb[:, ts(jt, H)], values_d[ts(jt, 128), :])

        # Manual ring of K=128 zero-padded chunk tiles. Rows 0-3 carry the
        # rank-4 operands; rows 4-127 stay zero so every preact matmul drives
        # the full 128-row array (keeps the PE activity monitor warm).
        NRING = 5
        krings = []
        qrings = []
        for r in range(NRING):
            kr = kch.tile([128, CH * Lk], BF16, tag=f"kring{r}")
            eng = nc.gpsimd if r % 2 == 0 else nc.vector
            eng.memset(kr[:, :], 0.0)
            nc.sync.dma_start(kr[0:2, :], ones_d[:, 0 : CH * Lk])
            krings.append(kr)
            qr = qch.tile([128, CH * 128], BF16, tag=f"qring{r}")
            nc.vector.memset(qr[:, :], 0.0)
            nc.sync.dma_start(qr[2:4, :], ones_d[:, 0 : CH * 128])
            qrings.append(qr)

        # ---- transposes: queries -> qsT (d, i); keys -> keysT (d, j) ----
        qsT_ps = ps_misc.tile([128, D], F32, tag="misc")
        for dt in range(2):
            nc.tensor.transpose(
                qsT_ps[:, ts(dt, 128)], queries_sb[:, ts(dt, 128)], ident_sb[:]
            )
        qsT_sb = proj.tile([128, D], F32)  # [d', dt*128 + i]
        nc.vector.tensor_copy(qsT_sb[:], qsT_ps[:])

        keysT_sb = proj.tile([128, 2 * Lk], F32)  # [d', dt*512 + jt*128 + j']
        for dt in range(2):
            keysT_ps = ps_pre.tile([128, GRP * Lk], F32, tag="pre")
            for jt in range(4):
                nc.tensor.transpose(
                    keysT_ps[:, ts(jt, 128)],
                    keys_sb[:, jt * D + dt * 128 : jt * D + dt * 128 + 128],
                    ident_sb[:],
                )
            nc.vector.tensor_copy(keysT_sb[:, ts(dt, Lk)], keysT_ps[:, 0:Lk])

        # ---- projections (fp32): qT (h, i), kT (h, j); split hi/lo bf16 ----
        qT_ps = ps_misc.tile([128, D], F32, tag="misc")
        for ht in range(2):
            for dt in range(2):
                nc.tensor.matmul(
                    qT_ps[:, ts(ht, 128)],
                    Wq_sb[:, dt * H + ht * 128 : dt * H + ht * 128 + 128],
                    qsT_sb[:, ts(dt, 128)],
                    start=(dt == 0),
                    stop=(dt == 1),
                )
        qhi_sb = proj.tile([128, D], BF16)  # [h', ht*128 + i]
        nc.vector.tensor_copy(qhi_sb[:], qT_ps[:])
        qlo_sb = proj.tile([128, D], BF16)
        nc.vector.tensor_sub(qlo_sb[:], qT_ps[:], qhi_sb[:])

        khi_sb = proj.tile([128, 2 * Lk], BF16)  # [h', ht*512 + j]
        klo_sb = proj.tile([128, 2 * Lk], BF16)
        for ht in range(2):
            kT_ps = ps_pre.tile([128, GRP * Lk], F32, tag="pre")
            for dt in range(2):
                nc.tensor.matmul(
                    kT_ps[:, 0:Lk],
                    Wk_sb[:, dt * H + ht * 128 : dt * H + ht * 128 + 128],
                    keysT_sb[:, ts(dt, Lk)],
                    start=(dt == 0),
                    stop=(dt == 1),
                )
            nc.vector.tensor_copy(khi_sb[:, ts(ht, Lk)], kT_ps[:, 0:Lk])
            nc.vector.tensor_sub(
                klo_sb[:, ts(ht, Lk)], kT_ps[:, 0:Lk], khi_sb[:, ts(ht, Lk)]
            )

        def row_src(sb, c, width):
            # rows h = c*CH .. c*CH+CH of the (h, x) matrix stored as
            # sb[(h % 128), (h // 128)*width + x] -> (CH, width) slice; the
            # DMA flattens it row-major into the chunk row (same total size)
            ht, r0 = divmod(c * CH, 128)
            return sb[r0 : r0 + CH, ts(ht, width)]

        loaded = {}

        def get_chunks(c):
            # kc rows: [ones, ones, k_hi, k_lo]; qc rows: [q_hi, q_lo, ones, ones]
            if c not in loaded:
                kc = krings[c % NRING]
                nc.sync.dma_start(kc[2:3, :], row_src(khi_sb, c, Lk))
                nc.sync.dma_start(kc[3:4, :], row_src(klo_sb, c, Lk))
                qc = qrings[c % NRING]
                nc.sync.dma_start(qc[0:1, :], row_src(qhi_sb, c, 128))
                nc.sync.dma_start(qc[1:2, :], row_src(qlo_sb, c, 128))
                loaded[c] = (qc, kc)
            return loaded[c]

        # ---- scores accumulator; masked init via rank-1 matmul ----
        sc_ps = ps_sc.tile([128, Lk], F32)
        nc.tensor.matmul(sc_ps[:], onesb_sb[:], mask_sb[:], start=True, stop=False)

        # ---- main loop over h in groups of GRP (software-pipelined) ----
        # PE FIFO order matters: preact matmuls for group g+2 are emitted
        # BEFORE the accumulate matmuls of group g, so the tanh for g+1 never
        # transitively waits on the DVE->accumulate chain.
        n_groups = (H + GRP - 1) // GRP
        n_chunks = H // CH
        pres = {}

        diags = {}

        def emit_pre(g):
            h0 = g * GRP
            sz = min(GRP, H - h0)
            for pc in range(h0 // CH, min((h0 + sz - 1) // CH + 2, n_chunks)):
                get_chunks(pc)
            dg3 = diagp.tile([128, GRP * 128], BF16, tag="dg")
            nc.gpsimd.dma_start(dg3[:], wvdiag_d[ts(g, 128), :])
            diags[g] = dg3
            pre = ps_pre.tile([128, GRP * Lk], F32, tag="pre")
            for l in range(sz):
                h = h0 + l
                c, hh = divmod(h, CH)
                qc, kc = get_chunks(c)
                nc.tensor.matmul(
                    pre[:, ts(l, Lk)],
                    qc[:, ts(hh, 128)],
                    kc[:, ts(hh, Lk)],
                    start=True,
                    stop=True,
                )
            pres[g] = pre

        emit_pre(0)
        emit_pre(1)
        for g in range(n_groups):
            h0 = g * GRP
            sz = min(GRP, H - h0)
            pre = pres.pop(g)
            feat = featp.tile([128, GRP * Lk], BF16, tag="feat")
            nc.scalar.activation(feat[:, 0 : sz * Lk], pre[:, 0 : sz * Lk], AF.Tanh)
            if g + 2 < n_groups:
                emit_pre(g + 2)
            dg3 = diags.pop(g)
            for l in range(sz):
                h = h0 + l
                nc.tensor.matmul(
                    sc_ps[:],
                    dg3[:, ts(l, 128)],
                    feat[:, ts(l, Lk)],
                    start=False,
                    stop=(h == H - 1),
                )

        # ---- masked softmax over j ----
        negmax = sm.tile([128, 1], F32)
        nc.vector.tensor_reduce(
            negmax[:],
            sc_ps[:],
            axis=mybir.AxisListType.X,
            op=mybir.AluOpType.max,
            negate=True,
        )
        p_sb = sm.tile([128, Lk], F32)
        sumexp = sm.tile([128, 1], F32)
        nc.scalar.activation(
            p_sb[:], sc_ps[:], AF.Exp, bias=negmax[:], accum_out=sumexp[:]
        )
        rinv = sm.tile([128, 1], F32)
        nc.vector.reciprocal(rinv[:], sumexp[:])

        # ---- attn @ values: transpose p, 4 accumulating fp32 matmuls ----
        pT_ps = ps_misc.tile([128, Lk], F32, tag="misc")
        for jt in range(4):
            nc.tensor.transpose(
                pT_ps[:, ts(jt, 128)], p_sb[:, ts(jt, 128)], ident_sb[:]
            )
        pT_sb = sm.tile([128, Lk], F32)  # [j', jt*128 + i]
        nc.vector.tensor_copy(pT_sb[:], pT_ps[:])

        out_ps = ps_misc.tile([128, H], F32, tag="misc")
        for jt in range(4):
            nc.tensor.matmul(
                out_ps[:],
                pT_sb[:, ts(jt, 128)],
                values_sb[:, ts(jt, H)],
                start=(jt == 0),
                stop=(jt == 3),
            )
        out_sb = sm.tile([128, H], F32)
        nc.vector.tensor_scalar_mul(out_sb[:], out_ps[:], rinv[:])
        nc.sync.dma_start(out_d[:], out_sb[:])

    nc.compile()
    return nc


def _get_program():
    if "nc" not in _CACHE:
        _CACHE["nc"] = build_program()
    return _CACHE["nc"]


def make_in_maps(queries, keys, values, valid_lens, Wq, Wk, wv):
    queries = np.ascontiguousarray(queries, dtype=np.float32)
    keys = np.ascontiguousarray(keys, dtype=np.float32)
    values = np.ascontiguousarray(values, dtype=np.float32)
    Wq = np.ascontiguousarray(Wq, dtype=np.float32)
    Wk = np.ascontiguousarray(Wk, dtype=np.float32)
    wv = np.ascontiguousarray(wv, dtype=np.float32).reshape(1, H)
    vl = np.asarray(valid_lens).astype(np.int64).reshape(NCORES)
    bf = ml_dtypes.bfloat16
    ones = np.ones((2, CH * Lk), dtype=bf)
    ident = np.eye(128, dtype=np.float32)
    identb = np.eye(128, dtype=bf)
    n_groups = H // GRP + 1
    wvdiag = np.zeros((n_groups, 128, GRP, 128), dtype=bf)
    idx = np.arange(128)
    for g in range(n_groups):
        for l in range(GRP):
            h = g * GRP + l
            if h < H:
                wvdiag[g, idx, l, idx] = bf(wv[0, h])
    wvdiag = wvdiag.reshape(n_groups * 128, GRP * 128)
    jj = np.arange(Lk)
    in_maps = []
    for b in range(NCORES):
        mask_b = np.where(jj >= vl[b], -1e6, 0.0).astype(bf)[None, :]
        in_maps.append(
            {
                "queries": queries[b],
                "keys": keys[b],
                "values": values[b],
                "Wq": Wq,
                "Wk": Wk,
                "wv": wv,
                "mask": mask_b,
                "ones": ones,
                "ident": ident,
                "identb": identb,
                "wvdiag": wvdiag,
            }
        )
    return in_maps


def kernel(**inputs):
    nc = _get_program()
    in_maps = make_in_maps(
        inputs["queries"],
        inputs["keys"],
        inputs["values"],
        inputs["valid_lens"],
        inputs["Wq"],
        inputs["Wk"],
        inputs["wv"],
    )
    res = run_bass_kernel_spmd(nc, in_maps, core_ids=list(range(NCORES)))
    out = np.stack([res.results[b]["out"] for b in range(NCORES)], axis=0)
    return out.astype(np.float32)


# revision 19
# speedup vs baseline: 1.5810x; 1.0029x over previous
"""AdditiveAttention on Trainium2 (Bass/Tile), data-parallel over batch across 8 cores.

Per-core problem (batch element b on core b):
  q = queries @ Wq                  (128, 256)
  k = keys @ Wk                     (512, 256)
  scores[i,j] = wv . tanh(q[i] + k[j])          (128, 512)
  masked softmax over j (j >= valid_len -> -1e6)
  out = attn @ values               (128, 256)

Kernel structure per core:
  - q/k projected in fp32, then split hi/lo into bf16 pairs (q ~ q_hi + q_lo)
  - PE K=4 bf16 matmuls build preact_h[i,j] = q[i,h] + k[j,h] in PSUM
    (rows: q_hi, q_lo, ones | ones, ones, k_hi, k_lo -> exact to ~2^-17)
  - ACT applies tanh on groups of 3 banks (PSUM -> bf16 SBUF)
  - DVE prescales feat by wv_h (bf16 4x mode, fp32 per-partition scalar)
  - PE accumulates scores += I.T @ (wv_h * feat_h) in PSUM (bf16 identity)
  - mask row added via rank-1 matmul (host-computed from valid_lens)
  - softmax: DVE reduce_max(neg) -> ACT exp(bias=-max, accum_out=sumexp) -> recip
  - PE transposes attn, 4 accumulating fp32 matmuls against values, row-scale by 1/sum
"""

import numpy as np
import ml_dtypes
from contextlib import ExitStack

from concourse import bacc, tile
import concourse.bass as bass
import concourse.mybir as mybir
from concourse.bass_utils import run_bass_kernel_spmd

F32 = mybir.dt.float32
BF16 = mybir.dt.bfloat16
AF = mybir.ActivationFunctionType
ts = bass.ts

Lq, Lk, D, H = 128, 512, 256, 256
NCORES = 8
CH = 8    # h-values per staged SBUF chunk
GRP = 3   # h-values per tanh group (3 PSUM banks)

_CACHE = {}


def build_program():
    nc = bacc.Bacc(
        "TRN2", target_bir_lowering=False, debug=False, enable_asserts=False
    )

    queries_d = nc.dram_tensor("queries", [Lq, D], F32, kind="ExternalInput")
    keys_d = nc.dram_tensor("keys", [Lk, D], F32, kind="ExternalInput")
    values_d = nc.dram_tensor("values", [Lk, H], F32, kind="ExternalInput")
    Wq_d = nc.dram_tensor("Wq", [D, H], F32, kind="ExternalInput")
    Wk_d = nc.dram_tensor("Wk", [D, H], F32, kind="ExternalInput")
    wv_d = nc.dram_tensor("wv", [1, H], F32, kind="ExternalInput")
    mask_d = nc.dram_tensor("mask", [1, Lk], BF16, kind="ExternalInput")
    ones_d = nc.dram_tensor("ones", [2, CH * Lk], BF16, kind="ExternalInput")
    ident_d = nc.dram_tensor("ident", [128, 128], F32, kind="ExternalInput")
    identb_d = nc.dram_tensor("identb", [128, 128], BF16, kind="ExternalInput")
    wvdiag_d = nc.dram_tensor("wvdiag", [(H // GRP + 1) * 128, GRP * 128], BF16, kind="ExternalInput")
    out_d = nc.dram_tensor("out", [Lq, H], F32, kind="ExternalOutput")

    with tile.TileContext(nc) as tc, ExitStack() as ctx:
        const = ctx.enter_context(tc.tile_pool(name="const", bufs=1))
        inp = ctx.enter_context(tc.tile_pool(name="inp", bufs=1))
        proj = ctx.enter_context(tc.tile_pool(name="proj", bufs=1))
        kch = ctx.enter_context(tc.tile_pool(name="kch", bufs=3))
        qch = ctx.enter_context(tc.tile_pool(name="qch", bufs=3))
        featp = ctx.enter_context(tc.tile_pool(name="featp", bufs=4))
        diagp = ctx.enter_context(tc.tile_pool(name="diagp", bufs=4))
        sm = ctx.enter_context(tc.tile_pool(name="sm", bufs=1))
        ps_pre = ctx.enter_context(tc.tile_pool(name="ps_pre", bufs=2, space="PSUM"))
        ps_sc = ctx.enter_context(tc.tile_pool(name="ps_sc", bufs=1, space="PSUM"))
        ps_misc = ctx.enter_context(tc.tile_pool(name="ps_misc", bufs=1, space="PSUM"))

        # ---- input loads (ident first: transposes need it; values last) ----
        ident_sb = const.tile([128, 128], F32)
        nc.sync.dma_start(ident_sb[:], ident_d[:])
        # preload the ACT spline tables (tanh/exp) off the critical path
        warm_sb = sm.tile([1, 2], F32)
        nc.scalar.activation(warm_sb[0:1, 0:1], ident_sb[0:1, 0:1], AF.Tanh)
        nc.scalar.activation(warm_sb[0:1, 1:2], ident_sb[0:1, 0:1], AF.Exp)
        queries_sb = inp.tile([128, D], F32)
        nc.sync.dma_start(queries_sb[:], queries_d[:])
        Wq_sb = inp.tile([128, 2 * H], F32)  # [d', dt*256 + h]
        for dt in range(2):
            nc.sync.dma_start(Wq_sb[:, ts(dt, H)], Wq_d[ts(dt, 128), :])
        keys_sb = inp.tile([128, 4 * D], F32)  # [j', jt*256 + d]
        for jt in range(4):
            nc.sync.dma_start(keys_sb[:, ts(jt, D)], keys_d[ts(jt, 128), :])
        Wk_sb = inp.tile([128, 2 * H], F32)
        for dt in range(2):
            nc.gpsimd.dma_start(Wk_sb[:, ts(dt, H)], Wk_d[ts(dt, 128), :])
        identb_sb = const.tile([128, 128], BF16)
        nc.gpsimd.dma_start(identb_sb[:], identb_d[:])
        mask_sb = const.tile([1, Lk], BF16)
        nc.gpsimd.dma_start(mask_sb[:], mask_d[:])
        onesb_sb = const.tile([1, 128], BF16)
        nc.gpsimd.dma_start(onesb_sb[:], ones_d[0:1, 0:128])
        values_sb = inp.tile([128, 4 * H], F32)  # [j', jt*256 + v]
        for jt in range(4):
            nc.gpsimd.dma_start(values_sb[:, ts(jt, H)], values_d[ts(jt, 128), :])

        # Manual ring of K=128 zero-padded chunk tiles. Rows 0-3 carry the
        # rank-4 operands; rows 4-127 stay zero so every preact matmul drives
        # the full 128-row array (keeps the PE activity monitor warm).
        NRING = 5
        krings = []
        qrings = []
        for r in range(NRING):
            kr = kch.tile([128, CH * Lk], BF16, tag=f"kring{r}")
            eng = nc.gpsimd if r % 2 == 0 else nc.vector
            eng.memset(kr[:, :], 0.0)
            nc.sync.dma_start(kr[0:2, :], ones_d[:, 0 : CH * Lk])
            krings.append(kr)
            qr = qch.tile([128, CH * 128], BF16, tag=f"qring{r}")
            nc.vector.memset(qr[:, :], 0.0)
            nc.sync.dma_start(qr[2:4, :], ones_d[:, 0 : CH * 128])
            qrings.append(qr)

        # ---- transposes: queries -> qsT (d, i); keys -> keysT (d, j) ----
        qsT_ps = ps_misc.tile([128, D], F32, tag="misc")
        for dt in range(2):
            nc.tensor.transpose(
                qsT_ps[:, ts(dt, 128)], queries_sb[:, ts(dt, 128)], ident_sb[:]
            )
        qsT_sb = proj.tile([128, D], F32)  # [d', dt*128 + i]
        nc.vector.tensor_copy(qsT_sb[:], qsT_ps[:])

        keysT_sb = proj.tile([128, 2 * Lk], F32)  # [d', dt*512 + jt*128 + j']
        for dt in range(2):
            keysT_ps = ps_pre.tile([128, GRP * Lk], F32, tag="pre")
            for jt in range(4):
                nc.tensor.transpose(
                    keysT_ps[:, ts(jt, 128)],
                    keys_sb[:, jt * D + dt * 128 : jt * D + dt * 128 + 128],
                    ident_sb[:],
                )
            nc.vector.tensor_copy(keysT_sb[:, ts(dt, Lk)], keysT_ps[:, 0:Lk])

        # ---- projections (fp32): qT (h, i), kT (h, j); split hi/lo bf16 ----
        qT_ps = ps_misc.tile([128, D], F32, tag="misc")
        for ht in range(2):
            for dt in range(2):
                nc.tensor.matmul(
                    qT_ps[:, ts(ht, 128)],
                    Wq_sb[:, dt * H + ht * 128 : dt * H + ht * 128 + 128],
                    qsT_sb[:, ts(dt, 128)],
                    start=(dt == 0),
                    stop=(dt == 1),
                )
        qhi_sb = proj.tile([128, D], BF16)  # [h', ht*128 + i]
        nc.vector.tensor_copy(qhi_sb[:], qT_ps[:])
        qlo_sb = proj.tile([128, D], BF16)
        nc.vector.tensor_sub(qlo_sb[:], qT_ps[:], qhi_sb[:])

        khi_sb = proj.tile([128, 2 * Lk], BF16)  # [h', ht*512 + j]
        klo_sb = proj.tile([128, 2 * Lk], BF16)
        for ht in range(2):
            kT_ps = ps_pre.tile([128, GRP * Lk], F32, tag="pre")
            for dt in range(2):
                nc.tensor.matmul(
                    kT_ps[:, 0:Lk],
                    Wk_sb[:, dt * H + ht * 128 : dt * H + ht * 128 + 128],
                    keysT_sb[:, ts(dt, Lk)],
                    start=(dt == 0),
                    stop=(dt == 1),
                )
            nc.vector.tensor_copy(khi_sb[:, ts(ht, Lk)], kT_ps[:, 0:Lk])
            nc.vector.tensor_sub(
                klo_sb[:, ts(ht, Lk)], kT_ps[:, 0:Lk], khi_sb[:, ts(ht, Lk)]
            )

        def row_src(sb, c, width):
            # rows h = c*CH .. c*CH+CH of the (h, x) matrix stored as
            # sb[(h % 128), (h // 128)*width + x] -> (CH, width) slice; the
            # DMA flattens it row-major into the chunk row (same total size)
            ht, r0 = divmod(c * CH, 128)
            return sb[r0 : r0 + CH, ts(ht, width)]

        loaded = {}

        def get_chunks(c):
            # kc rows: [ones, ones, k_hi, k_lo]; qc rows: [q_hi, q_lo, ones, ones]
            if c not in loaded:
                kc = krings[c % NRING]
                nc.sync.dma_start(kc[2:3, :], row_src(khi_sb, c, Lk))
                nc.sync.dma_start(kc[3:4, :], row_src(klo_sb, c, Lk))
                qc = qrings[c % NRING]
                nc.sync.dma_start(qc[0:1, :], row_src(qhi_sb, c, 128))
                nc.sync.dma_start(qc[1:2, :], row_src(qlo_sb, c, 128))
                loaded[c] = (qc, kc)
            return loaded[c]

        # ---- scores accumulator; masked init via rank-1 matmul ----
        sc_ps = ps_sc.tile([128, Lk], F32)
        nc.tensor.matmul(sc_ps[:], onesb_sb[:], mask_sb[:], start=True, stop=False)

        # ---- main loop over h in groups of GRP (software-pipelined) ----
        # PE FIFO order matters: preact matmuls for group g+2 are emitted
        # BEFORE the accumulate matmuls of group g, so the tanh for g+1 never
        # transitively waits on the DVE->accumulate chain.
        n_groups = (H + GRP - 1) // GRP
        n_chunks = H // CH
        pres = {}

        diags = {}

        def emit_pre(g):
            h0 = g * GRP
            sz = min(GRP, H - h0)
            for pc in range(h0 // CH, min((h0 + sz - 1) // CH + 2, n_chunks)):
                get_chunks(pc)
            dg3 = diagp.tile([128, GRP * 128], BF16, tag="dg")
            nc.gpsimd.dma_start(dg3[:], wvdiag_d[ts(g, 128), :])
            diags[g] = dg3
            pre = ps_pre.tile([128, GRP * Lk], F32, tag="pre")
            for l in range(sz):
                h = h0 + l
                c, hh = divmod(h, CH)
                qc, kc = get_chunks(c)
                nc.tensor.matmul(
                    pre[:, ts(l, Lk)],
                    qc[:, ts(hh, 128)],
                    kc[:, ts(hh, Lk)],
                    start=True,
                    stop=True,
                )
            pres[g] = pre

        feats = {}

        def emit_acc(g):
            # accumulate group g (its feat finished during the previous
            # group's tanh, so these matmuls never stall the PE FIFO)
            h0 = g * GRP
            sz = min(GRP, H - h0)
            feat = feats.pop(g)
            dg3 = diags.pop(g)
            for l in range(sz):
                h = h0 + l
                nc.tensor.matmul(
                    sc_ps[:],
                    dg3[:, ts(l, 128)],
                    feat[:, ts(l, Lk)],
                    start=False,
                    stop=(h == H - 1),
                )

        emit_pre(0)
        emit_pre(1)
        for g in range(n_groups):
            h0 = g * GRP
            sz = min(GRP, H - h0)
            pre = pres.pop(g)
            feat = featp.tile([128, GRP * Lk], BF16, tag="feat")
            nc.scalar.activation(feat[:, 0 : sz * Lk], pre[:, 0 : sz * Lk], AF.Tanh)
            feats[g] = feat
            if g >= 1:
                emit_acc(g - 1)
            if g + 2 < n_groups:
                emit_pre(g + 2)
        emit_acc(n_groups - 1)

        # ---- masked softmax over j ----
        negmax = sm.tile([128, 1], F32)
        nc.vector.tensor_reduce(
            negmax[:],
            sc_ps[:],
            axis=mybir.AxisListType.X,
            op=mybir.AluOpType.max,
            negate=True,
        )
        p_sb = sm.tile([128, Lk], F32)
        sumexp = sm.tile([128, 1], F32)
        nc.scalar.activation(
            p_sb[:], sc_ps[:], AF.Exp, bias=negmax[:], accum_out=sumexp[:]
        )
        rinv = sm.tile([128, 1], F32)
        nc.vector.reciprocal(rinv[:], sumexp[:])

        # ---- attn @ values: transpose p, 4 accumulating fp32 matmuls ----
        pT_ps = ps_misc.tile([128, Lk], F32, tag="misc")
        for jt in range(4):
            nc.tensor.transpose(
                pT_ps[:, ts(jt, 128)], p_sb[:, ts(jt, 128)], ident_sb[:]
            )
        pT_sb = sm.tile([128, Lk], F32)  # [j', jt*128 + i]
        nc.vector.tensor_copy(pT_sb[:], pT_ps[:])

        out_ps = ps_misc.tile([128, H], F32, tag="misc")
        for jt in range(4):
            nc.tensor.matmul(
                out_ps[:],
                pT_sb[:, ts(jt, 128)],
                values_sb[:, ts(jt, H)],
                start=(jt == 0),
                stop=(jt == 3),
            )
        out_sb = sm.tile([128, H], F32)
        nc.vector.tensor_scalar_mul(out_sb[:], out_ps[:], rinv[:])
        nc.sync.dma_start(out_d[:], out_sb[:])

    nc.compile()
    return nc


def _get_program():
    if "nc" not in _CACHE:
        _CACHE["nc"] = build_program()
    return _CACHE["nc"]


def make_in_maps(queries, keys, values, valid_lens, Wq, Wk, wv):
    queries = np.ascontiguousarray(queries, dtype=np.float32)
    keys = np.ascontiguousarray(keys, dtype=np.float32)
    values = np.ascontiguousarray(values, dtype=np.float32)
    Wq = np.ascontiguousarray(Wq, dtype=np.float32)
    Wk = np.ascontiguousarray(Wk, dtype=np.float32)
    wv = np.ascontiguousarray(wv, dtype=np.float32).reshape(1, H)
    vl = np.asarray(valid_lens).astype(np.int64).reshape(NCORES)
    bf = ml_dtypes.bfloat16
    ones = np.ones((2, CH * Lk), dtype=bf)
    ident = np.eye(128, dtype=np.float32)
    identb = np.eye(128, dtype=bf)
    n_groups = H // GRP + 1
    wvdiag = np.zeros((n_groups, 128, GRP, 128), dtype=bf)
    idx = np.arange(128)
    for g in range(n_groups):
        for l in range(GRP):
            h = g * GRP + l
            if h < H:
                wvdiag[g, idx, l, idx] = bf(wv[0, h])
    wvdiag = wvdiag.reshape(n_groups * 128, GRP * 128)
    jj = np.arange(Lk)
    in_maps = []
    for b in range(NCORES):
        mask_b = np.where(jj >= vl[b], -1e6, 0.0).astype(bf)[None, :]
        in_maps.append(
            {
                "queries": queries[b],
                "keys": keys[b],
                "values": values[b],
                "Wq": Wq,
                "Wk": Wk,
                "wv": wv,
                "mask": mask_b,
                "ones": ones,
                "ident": ident,
                "identb": identb,
                "wvdiag": wvdiag,
            }
        )
    return in_maps


def kernel(**inputs):
    nc = _get_program()
    in_maps = make_in_maps(
        inputs["queries"],
        inputs["keys"],
        inputs["values"],
        inputs["valid_lens"],
        inputs["Wq"],
        inputs["Wk"],
        inputs["wv"],
    )
    res = run_bass_kernel_spmd(nc, in_maps, core_ids=list(range(NCORES)))
    out = np.stack([res.results[b]["out"] for b in range(NCORES)], axis=0)
    return out.astype(np.float32)
